# revision 27
# baseline (speedup 1.0000x reference)
"""Trainium2 Bass kernel for CLSProcess: diagonal linear recurrence
state_t = y_t * state_{t-1} + x_t * z_t over [B=8, T=4096, units=1024].

Sharding: batch across the 8 cores (one batch element per core); the
recurrence is handled per-core with a chunked scan:
  - time is cut into 32 blocks of L=128 steps (partition dim = time)
  - per block, the decay matrix M'[t,s] = x_s * prod_{r=s+1..t} y_r
    (0 for s>t) is built EXACTLY with a DVE tensor_tensor_scan whose
    injection tensor is a host-staged diagonal xdiag[p,t] =
    x_t * I[p==t mod 128] (a zero-FLOP sparse re-layout of x):
    state_s(t) = y_t*state + xdiag  =>  out[s,t] = M'[t,s], the lhsT
    layout the PE matmul wants, with x already folded in. Scans are
    batched 4 blocks per instruction ([128,512]) with the y at block
    boundaries zeroed so the running state resets at each block start.
  - block output = M' @ z  (PE matmul, bf16 operands, fp32 PSUM) +
    carry term
  - carry term: po[t,:] += p_t * prev[127,:] where the within-block
    decay products p_t = prod_{r=block_start..t} y_r arrive as a tiny
    host-staged fp32 row (per-block cumprods of y, O(T) prep), so sel
    needs just one broadcast + one masked copy per 4-block group.
    Engines can only address partition bases {0,32,64,96} and matmul
    bases {0,32,64}, so row 127 of the previous block is reached with
    a K=64 matmul: sel[s,t] = I[s==127] * p_t (rows [64:128] used)
    against prev[64:128,:].
  - scheduling: block k+1's main matmuls are emitted BEFORE block k's
    carry matmuls so the in-order PE queue is never head-of-line
    blocked by the serial carry chain (PSUM pool is 4 deep = all 8
    banks). PSUM drains are split by the free dim across the scalar
    and vector engines. (Deeper lookahead, batched DMAs, and finer
    drain splits were all measured SLOWER - this emission shape is a
    sharp local optimum for the tile scheduler.)
  - I/O is bf16, halving HBM traffic; y and the p-row ride along as a
    small fp32 sidecar so the decay products stay full precision. The
    host upcasts the bf16 result to fp32.
"""

import numpy as np
import ml_dtypes

import concourse.bacc as bacc
import concourse.bass as bass
import concourse.mybir as mybir
import concourse.tile as tile
from concourse.bass_utils import run_bass_kernel_spmd

B = 8
T = 4096
F = 1026
U = 1024
L = 128
G = 4  # blocks per scan batch
D = 1  # blocks per DMA batch
NB = T // L  # 32 blocks
NG = NB // G  # 8 scan groups
ND = NB // D  # 32 DMA groups
CW = 64  # carry matmul contraction width (matmul bases must be 0/32/64)
SPLIT = 512  # drain free-dim split point: scalar [0:SPLIT], vector [SPLIT:]
DEPTH = 1  # main-matmul emission lookahead (blocks) over the carry chain
f32 = mybir.dt.float32
bf16 = mybir.dt.bfloat16
nbf16 = ml_dtypes.bfloat16


def build_nc() -> bass.Bass:
    nc = bacc.Bacc()
    # zin[d, p, c] = z_{d*L + p, c}  (identical memory layout to [T, U])
    zin = nc.dram_tensor("zin", [ND, L, D * U], bf16, kind="ExternalInput")
    # yz[0, t] = y_t with block-start entries zeroed (scan reset)
    yz = nc.dram_tensor("yz", [1, T], f32, kind="ExternalInput")
    # xdiag[g, p, j*L+i] = x_{g*G*L+j*L+i} * I[p==i]: scan injection
    xdiag = nc.dram_tensor("xdiag", [NG, L, G * L], bf16, kind="ExternalInput")
    # prow[0, k*L+i] = prod_{r=k*L..k*L+i} y_r: per-block decay products
    prow = nc.dram_tensor("prow", [1, T], f32, kind="ExternalInput")
    out = nc.dram_tensor("out", [ND, L, D * U], bf16, kind="ExternalOutput")

    e127_np = np.zeros((L, 1), dtype=np.float32)
    e127_np[L - 1, 0] = 1.0
    e127_d = nc.inline_tensor(e127_np, name="e127")

    with tile.TileContext(nc) as tc:
        with (
            tc.tile_pool(name="const", bufs=1) as constp,
            tc.tile_pool(name="zpool", bufs=8) as zpool,
            tc.tile_pool(name="dgpool", bufs=2) as dgp,
            tc.tile_pool(name="mtpool", bufs=2) as mtp,
            tc.tile_pool(name="ybcpool", bufs=2) as ybcp,
            tc.tile_pool(name="pbcpool", bufs=2) as pbcp,
            tc.tile_pool(name="selpool", bufs=2) as selp,
            tc.tile_pool(name="otpool", bufs=6) as otp,
            tc.tile_pool(name="ps_out", bufs=4, space="PSUM") as psp,
        ):
            e127 = constp.tile([L, 1], f32, tag="e127")
            nc.sync.dma_start(e127[:], e127_d[:, :])
            yzfull = constp.tile([1, T], f32, tag="yz")
            nc.sync.dma_start(yzfull[:], yz[0:1, :])
            prowfull = constp.tile([1, T], f32, tag="prow")
            nc.sync.dma_start(prowfull[:], prow[0:1, :])

            # group-level prep: one scan (vector), two broadcasts (gpsimd),
            # one masked copy (scalar) per 4-block group
            mt4s, sel4s = {}, {}

            def prep_group(g):
                c0 = g * G * L
                dg = dgp.tile([L, G * L], bf16, tag="dg")
                nc.sync.dma_start(dg[:], xdiag[g, :, :])
                ybc4 = ybcp.tile([L, G * L], f32, tag="ybc4")
                nc.gpsimd.partition_broadcast(
                    ybc4[:], yzfull[0:1, c0 : c0 + G * L]
                )
                # mt4[s, L*j + t] = x_s * prod_{r=s+1..t} y_r  (block g*G+j)
                mt4 = mtp.tile([L, G * L], bf16, tag="mt4")
                nc.vector.tensor_tensor_scan(
                    mt4[:],
                    ybc4[:],
                    dg[:],
                    0.0,
                    mybir.AluOpType.mult,
                    mybir.AluOpType.add,
                )
                # sel4[s, L*j + t] = I[s==127] * p_t  (p from the host row,
                # independent of the scan)
                pbc4 = pbcp.tile([L, G * L], f32, tag="pbc4")
                nc.gpsimd.partition_broadcast(
                    pbc4[:], prowfull[0:1, c0 : c0 + G * L]
                )
                sel4 = selp.tile([L, G * L], bf16, tag="sel4")
                nc.scalar.activation(
                    sel4[:],
                    pbc4[:],
                    mybir.ActivationFunctionType.Copy,
                    scale=e127[:],
                )
                mt4s[g], sel4s[g] = mt4, sel4

            # per-block state for the software-pipelined emission
            tzs, pos, ots = {}, {}, {}

            def emit_load(k):
                if k % D == 0:
                    d = k // D
                    tz = zpool.tile([L, D * U], bf16, tag="tz")
                    nc.sync.dma_start(tz[:], zin[d, :, :])
                    for jj in range(D):
                        tzs[k + jj] = tz[:, jj * U : (jj + 1) * U]

            def emit_main(k):
                g, j = k // G, k % G
                if j == 0:
                    prep_group(g)
                po = psp.tile([L, U], f32, tag="po")
                for jj in (0, 512):
                    nc.tensor.matmul(
                        po[:, jj : jj + 512],
                        mt4s[g][:, L * j : L * (j + 1)],
                        tzs[k][:, jj : jj + 512],
                        start=True,
                        stop=(k == 0),
                    )
                pos[k] = po

            def emit_carry_and_drain(k):
                g, j = k // G, k % G
                po = pos.pop(k)
                if k > 0:
                    prev = ots[k - 1]
                    # po[t, :] += p_t * prev[127, :]
                    for jj in (0, 512):
                        nc.tensor.matmul(
                            po[:, jj : jj + 512],
                            sel4s[g][L - CW : L, L * j : L * (j + 1)],
                            prev[L - CW : L, jj : jj + 512],
                            start=False,
                            stop=True,
                        )
                if k % D == 0:
                    d = k // D
                    ot = otp.tile([L, D * U], bf16, tag="ot")
                    for jj in range(D):
                        ots[k + jj] = ot[:, jj * U : (jj + 1) * U]
                    ots[(d, "tile")] = ot
                otk = ots[k]
                # drain split by the free dim: one piece per engine
                nc.scalar.copy(otk[:, 0:SPLIT], po[:, 0:SPLIT])
                nc.vector.tensor_copy(otk[:, SPLIT:U], po[:, SPLIT:U])
                if k % D == D - 1:
                    d = k // D
                    nc.sync.dma_start(out[d, :, :], ots.pop((d, "tile"))[:])

            # software pipeline: mains run DEPTH blocks ahead of carries
            for k in range(DEPTH):
                emit_load(k)
                emit_main(k)
            for k in range(DEPTH, NB):
                emit_load(k)
                emit_main(k)
                emit_carry_and_drain(k - DEPTH)
            for k in range(NB - DEPTH, NB):
                emit_carry_and_drain(k)
    nc.finalize()
    return nc


_NC = None


def _get_nc() -> bass.Bass:
    global _NC
    if _NC is None:
        _NC = build_nc()
    return _NC


def prep_in_maps(x: np.ndarray) -> list[dict]:
    xs = x[:, :, 0]  # [B,T]
    ys = x[:, :, 1]  # [B,T]
    zb = (
        np.ascontiguousarray(x[:, :, 2:])
        .astype(nbf16)
        .reshape(B, ND, D, L, U)
        .transpose(0, 1, 3, 2, 4)
        .reshape(B, ND, L, D * U)
    )
    zb = np.ascontiguousarray(zb)

    mask0 = (np.arange(T) % L) == 0
    yz = np.where(mask0[None, :], 0.0, ys).astype(np.float32)[:, None, :]

    # sparse re-layout of x for the scan injection (zero FLOPs)
    idx = np.arange(T)
    xdiag = np.zeros((B, NG, L, G * L), dtype=nbf16)
    xdiag[:, idx // (G * L), idx % L, idx % (G * L)] = xs[:, idx].astype(nbf16)

    # per-block decay products of y (O(T) host prep)
    prow = (
        ys.reshape(B, NB, L).cumprod(axis=2, dtype=np.float32)
        .reshape(B, 1, T)
        .astype(np.float32)
    )

    return [
        {"zin": zb[c], "yz": yz[c], "xdiag": xdiag[c], "prow": prow[c]}
        for c in range(B)
    ]


def unpack_out(outb: np.ndarray) -> np.ndarray:
    # [B, ND, L, D*U] -> [B, T, U]
    return (
        outb.reshape(B, ND, L, D, U)
        .transpose(0, 1, 3, 2, 4)
        .reshape(B, T, U)
        .astype(np.float32)
    )


def kernel(**inputs: np.ndarray) -> np.ndarray:
    x = np.ascontiguousarray(inputs["inputs"], dtype=np.float32)
    assert x.shape == (B, T, F), x.shape
    nc = _get_nc()
    res = run_bass_kernel_spmd(nc, prep_in_maps(x), core_ids=list(range(B)))
    outb = np.stack([res.results[c]["out"] for c in range(B)], axis=0)
    return unpack_out(outb)
